# revision 1
# baseline (speedup 1.0000x reference)
"""Trainium2 Bass kernel for ChebyshevActivation.

Math:
    scale = clip(input_scale, 0.1, 2.0)
    t = tanh(x * scale)                        # t in (-1, 1)
    out[b, o] = sum_w coeffs[o, w] * sum_i T_w(t[b, i])

Since |t| < 1, all Chebyshev T_n(t) lie in [-1, 1] and the reference's
clip(+-100) is dead code.  We work in the monomial basis: with power sums
M_j[b] = sum_i t[b,i]^j (M_0 = IN_F exactly) and G = coeffs @ C (C the
Chebyshev->monomial matrix), out = M @ G^T.

Power-sum extraction is one fused pass per "piece": ACT squares with
accum_out, or DVE scalar_tensor_tensor with accum_out (this environment's
walrus rejects TensorScalarPtr on GPSIMD and raw-ISA custom-DVE encodings
from plain Bass, so pieces live on ACT/DVE and the module is built with
Bacc).  Every moment can be split column-wise into pieces on different
engines; each piece accumulates into its own column of the per-tile moment
matrix, and the host duplicates the matching G rows so the final PE matmul
(K = #pieces+1) re-merges them.  Channels t1..t4 are fp16 (bf16 loses too
much precision through the basis change; fp16 keeps DVE 2x modes).

Per-core layout: data-parallel over batch, 8 cores x 1024 rows,
8 row-tiles of [128, 2048] per core.
"""

import numpy as np

import concourse.bass as bass
import concourse.bacc as bacc
import concourse.mybir as mybir
import concourse.tile as tile
from concourse import masks
from concourse.bass_utils import run_bass_kernel_spmd

# This environment's walrus build rejects raw client-encoded ISA instructions
# ("ISA wrong length" for the 64-byte EVENT_SEMAPHORE_RANGE_CLEAR emitted by
# the TileContext exit barrier).  Replace the range-clear with per-semaphore
# EventSemaphore writes (update_mode=sem-wr-imm, value 0), which this walrus
# accepts, so re-executing the loaded NEFF still sees cleared semaphores.
def _sem_clear_via_events(self, sem_range):
    # Spread the writes across all engines so they retire in parallel between
    # the two exit barriers instead of serially on GPSIMD.
    engines = list(self.bass.engines.values())
    inst = None
    for i, s in enumerate(sem_range):
        eng = engines[i % len(engines)]
        inst = mybir.InstEventSemaphore(
            name=self.bass.get_next_instruction_name(),
            ins=[], outs=[],
            sync_info=mybir.SyncInfo(
                on_wait=[],
                on_update=[mybir.SyncUpdate(
                    sync_type="semaphore", id=s,
                    update_mode="sem-wr-imm", update_value=0,
                )],
            ),
        )
        eng.add_instruction(inst)
    return inst


bass.BassGpSimd.sem_clear = _sem_clear_via_events

N_CORES = 8
BATCH = 8192
IN_F = 2048
OUT_F = 1024
DEG = 8
W = DEG + 1  # 9 moments
ROWS_PER_CORE = BATCH // N_CORES  # 1024
P = 128
NTILES = ROWS_PER_CORE // P  # 8

F32 = mybir.dt.float32
F16 = mybir.dt.float16
MULT = mybir.AluOpType.mult
ADD = mybir.AluOpType.add
SQUARE = mybir.ActivationFunctionType.Square
TANH = mybir.ActivationFunctionType.Tanh

# Stream definitions: name -> (in0, in1, value_dst) ; value_dst None => junk.
# in0 == in1 means the stream is a square (ACT-eligible).
STREAMS = {
    "t2": ("t1", "t1", "t2"),
    "t3": ("t2", "t1", "t3"),
    "t4": ("t2", "t2", "t4"),
    "M6": ("t3", "t3", None),
    "M5": ("t4", "t1", None),
    "M8": ("t4", "t4", None),
    "M7": ("t4", "t3", None),
}
STREAM_MOMENT = {"t2": 2, "t3": 3, "t4": 4, "M5": 5, "M6": 6, "M7": 7, "M8": 8}

# Engine assignment config: stream -> list of (engine, fraction).
# Engines: "A" = ACT square (squares only), "D" = DVE TTR, "G" = GPSIMD stt.
CFG = {
    "t2": [("D", 1.0)],
    "t3": [("D", 1.0)],
    "t4": [("A", 1.0)],
    "M5": [("D", 1.0)],
    "M6": [("A", 1.0)],
    "M7": [("D", 1.0)],
    "M8": [("A", 1.0)],
    "oc_act": 1.0,   # fraction of the PSUM->SBUF output copy done on ACT
    "mt": "D",       # moment-transpose PSUM->SBUF copy engine
    "xin_bufs": 4,
    "chan_bufs": 3,
    "t1_bufs": 3,
    "ostage_bufs": 3,
    # warm-up: chunk tile 0's tanh/t2/t3 into column halves (extra partial-
    # moment columns, re-merged by duplicated G rows) so DVE starts ~2us sooner
    "warm": True,
    "warm_set": ("t2", "t3"),
    # deprioritize the ACT output copy so the next tile's critical squares
    # (which DVE's cross-product streams wait on) win the scheduling race
    "oc_prio_bump": 32,
    # last tile: split the output copy across ACT+DVE halves with two
    # pipelined DMA-outs to shorten the tail chain
    "tail_fast": True,
}


def _cheb_monomial_matrix(deg=DEG):
    C = np.zeros((deg + 1, deg + 1), dtype=np.float64)
    C[0, 0] = 1.0
    if deg >= 1:
        C[1, 1] = 1.0
    for n in range(2, deg + 1):
        C[n, 1:] = 2.0 * C[n - 1, :-1]
        C[n, :] -= C[n - 2, :]
    return C


def _pieces(cfg):
    """Deterministic piece list: (stream, engine, col_lo, col_hi)."""
    out = []
    enabled = cfg.get("only_streams")
    for s in STREAMS:
        if enabled is not None and s not in enabled:
            continue
        cols = 0
        parts = cfg[s]
        for idx, (eng, frac) in enumerate(parts):
            if idx == len(parts) - 1:
                hi = IN_F
            else:
                hi = cols + int(round(IN_F * frac / 128.0)) * 128
                hi = min(hi, IN_F)
            if hi > cols:
                out.append((s, eng, cols, hi))
            cols = hi
    return out


def _moment_rows(cfg):
    """Row j of GT corresponds to these moments: [0 (M0), 1 (M1 tanh), *pieces,
    then warm-up duplicate rows for tile 0's chunked t1/t2/t3/t4 streams]."""
    rows = [0, 1]
    for s, _eng, _lo, _hi in _pieces(cfg):
        rows.append(STREAM_MOMENT[s])
    if cfg.get("warm"):
        rows += [1, 2, 3, 4]
    return rows



def _emit_out(nc, cfg, oc_pair, pout, ostage, mt_sb, gt_sb, out, it):
    if oc_pair:
        if not hasattr(nc, "_ocp_state") or it % 2 == 0:
            nc._ocp_state = (
                pout.tile([P, 2 * OUT_F], F32, tag="opp"),
                ostage.tile([P, 2 * OUT_F], F32, tag="osp"),
            )
        o_ps_pair, o_sb_pair = nc._ocp_state
        base = (it % 2) * OUT_F
        for h in range(2):
            nc.tensor.matmul(
                o_ps_pair[:, base + h * 512:base + (h + 1) * 512],
                lhsT=mt_sb[:, :],
                rhs=gt_sb[:, h * 512:(h + 1) * 512],
                start=True, stop=True,
            )
        if it % 2 == 1:
            nc.scalar.copy(o_sb_pair[:, :], o_ps_pair[:, :])
            for s in range(2):
                it0 = it - 1 + s
                nc.sync.dma_start(
                    out=out[it0 * P:(it0 + 1) * P, :],
                    in_=o_sb_pair[:, s * OUT_F:(s + 1) * OUT_F],
                )
    else:
        o_ps = pout.tile([P, OUT_F], F32)
        for h in range(2):
            nc.tensor.matmul(
                o_ps[:, h * 512:(h + 1) * 512],
                lhsT=mt_sb[:, :],
                rhs=gt_sb[:, h * 512:(h + 1) * 512],
                start=True, stop=True,
            )
        o_sb = ostage.tile([P, OUT_F], F32)
        if cfg.get("tail_fast") and it == NTILES - 1:
            # split the last tile's output copy across ACT+DVE in parallel
            # halves, each followed by its own DMA, to shorten the tail chain
            H2 = OUT_F // 2
            nc.scalar.copy(o_sb[:, 0:H2], o_ps[:, 0:H2])
            nc.vector.tensor_copy(o_sb[:, H2:OUT_F], o_ps[:, H2:OUT_F])
            nc.sync.dma_start(out=out[it * P:(it + 1) * P, 0:H2],
                              in_=o_sb[:, 0:H2])
            nc.sync.dma_start(out=out[it * P:(it + 1) * P, H2:OUT_F],
                              in_=o_sb[:, H2:OUT_F])
            return
        ca = int(round(OUT_F * cfg["oc_act"] / 128.0)) * 128
        ca = max(0, min(OUT_F, ca))
        if cfg.get("oc_last_dve") and it == NTILES - 1:
            ca = 0
        ocb = cfg.get("oc_prio_bump", 0)
        if ca > 0:
            r = nc.scalar.copy(o_sb[:, 0:ca], o_ps[:, 0:ca])
            if ocb:
                r.ins.bass_priority += ocb
        if ca < OUT_F:
            r = nc.vector.tensor_copy(o_sb[:, ca:OUT_F], o_ps[:, ca:OUT_F])
            if ocb:
                r.ins.bass_priority += ocb
        nc.sync.dma_start(out=out[it * P:(it + 1) * P, :], in_=o_sb[:, :])


def _build_nc(scale: float, cfg=CFG) -> bass.Bass:
    pieces = _pieces(cfg)
    warm = bool(cfg.get("warm"))
    K = 2 + len(pieces) + (4 if warm else 0)  # M0 + M1 + pieces [+ warm dups]
    assert K <= 24
    mcols = K

    nc = bacc.Bacc("TRN2")
    x = nc.dram_tensor("x", [ROWS_PER_CORE, IN_F], F32, kind="ExternalInput")
    gt = nc.dram_tensor("gt", [K, OUT_F], F32, kind="ExternalInput")
    out = nc.dram_tensor("out", [ROWS_PER_CORE, OUT_F], F32, kind="ExternalOutput")

    oc_pair = cfg.get("oc_pair", False)
    with tile.TileContext(nc) as tc:
        with (
            tc.tile_pool(name="singles", bufs=1) as singles,
            tc.tile_pool(name="xin", bufs=cfg["xin_bufs"]) as xin,
            tc.tile_pool(name="chan", bufs=cfg["chan_bufs"]) as chan,
            tc.tile_pool(name="chan1", bufs=cfg.get("t1_bufs", cfg["chan_bufs"])) as chan1,
            tc.tile_pool(name="junk", bufs=1) as junkp,
            tc.tile_pool(name="mpool", bufs=4) as mpool,
            tc.tile_pool(name="mtsb", bufs=4) as mtsb,
            tc.tile_pool(name="ostage", bufs=cfg["ostage_bufs"]) as ostage,
            tc.tile_pool(name="pt", bufs=cfg.get("pt_bufs", 2), space="PSUM") as pt,
            tc.tile_pool(name="pout", bufs=(1 if oc_pair else cfg.get("pout_bufs", 2)),
                         space="PSUM") as pout,
        ):
            if cfg.get("mt_batch", 1) > 1:
                gt_sb = singles.tile([32 + K, OUT_F], F32)
                nc.sync.dma_start(out=gt_sb[0:K, :], in_=gt[:, :])
                nc.sync.dma_start(out=gt_sb[32:32 + K, :], in_=gt[:, :])
            else:
                gt_sb = singles.tile([K, OUT_F], F32)
                nc.sync.dma_start(out=gt_sb[:, :], in_=gt[:, :])
            ident = singles.tile([P, P], F32)
            masks.make_identity(nc, ident[:, :])

            j_dve = junkp.tile([P, IN_F], F16, tag="jd")
            j_act = junkp.tile([P, IN_F], F16, tag="ja")
            j_gps = junkp.tile([P, IN_F], F16, tag="jg")
            JUNK = {"A": j_act, "D": j_dve, "G": j_gps}

            mt_batch = cfg.get("mt_batch", 1)
            m_pair = None
            for it in range(NTILES):
                x_t = xin.tile([P, IN_F], F32)
                chunked = warm and it == 0
                H = IN_F // 2
                if chunked:
                    nc.sync.dma_start(out=x_t[:, 0:H], in_=x[it * P:(it + 1) * P, 0:H])
                    nc.sync.dma_start(out=x_t[:, H:IN_F], in_=x[it * P:(it + 1) * P, H:IN_F])
                else:
                    nc.sync.dma_start(out=x_t[:, :], in_=x[it * P:(it + 1) * P, :])

                if mt_batch > 1:
                    if it % mt_batch == 0:
                        m_pair = mpool.tile([P, mt_batch * 32], F32, tag="mp")
                    m_t = m_pair[:, (it % mt_batch) * 32:(it % mt_batch) * 32 + mcols]
                else:
                    m_t = mpool.tile([P, mcols], F32)
                nc.gpsimd.memset(m_t[:, 0:1], float(IN_F))

                t1 = chan1.tile([P, IN_F], F16, tag="t1")
                t2 = chan.tile([P, IN_F], F16, tag="t2")
                t3 = chan.tile([P, IN_F], F16, tag="t3")
                t4 = chan.tile([P, IN_F], F16, tag="t4")
                VALS = {"t1": t1, "t2": t2, "t3": t3, "t4": t4}

                if warm and not chunked:
                    nc.gpsimd.memset(m_t[:, K - 4:K], 0.0)
                elif warm and chunked:
                    # zero warm columns whose stream is not chunked on tile 0
                    wset = cfg.get("warm_set", ("t2", "t3", "t4"))
                    for nm, off in (("t2", 1), ("t3", 2), ("t4", 3)):
                        if nm not in wset:
                            nc.gpsimd.memset(m_t[:, K - 4 + off:K - 3 + off], 0.0)

                # t1 = tanh(scale * x), accum -> M1 (col 1; chunk b -> warm col)
                if chunked:
                    nc.scalar.activation(
                        out=t1[:, 0:H], in_=x_t[:, 0:H], func=TANH,
                        scale=scale, accum_out=m_t[:, 1:2],
                    )
                    nc.scalar.activation(
                        out=t1[:, H:IN_F], in_=x_t[:, H:IN_F], func=TANH,
                        scale=scale, accum_out=m_t[:, K - 4:K - 3],
                    )
                else:
                    nc.scalar.activation(
                        out=t1[:, :], in_=x_t[:, :], func=TANH,
                        scale=scale, accum_out=m_t[:, 1:2],
                    )

                for pidx, (s, eng, lo, hi) in enumerate(pieces):
                    a_name, b_name, dst_name = STREAMS[s]
                    a = VALS[a_name]
                    b = VALS[b_name]
                    dst = VALS[dst_name] if dst_name else JUNK[eng]
                    mcol = m_t[:, 2 + pidx:3 + pidx]
                    if (chunked and s in cfg.get("warm_set", ("t2", "t3", "t4"))
                            and lo == 0 and hi == IN_F):
                        # split tile-0 value streams; 2nd chunk accums into warm col
                        wcol_i = K - 4 + {"t2": 1, "t3": 2, "t4": 3}[s]
                        wcol = m_t[:, wcol_i:wcol_i + 1]
                        for (clo, chi, mc) in ((0, H, mcol), (H, IN_F, wcol)):
                            if eng == "A":
                                nc.scalar.activation(
                                    out=dst[:, clo:chi], in_=a[:, clo:chi],
                                    func=SQUARE, accum_out=mc,
                                )
                            else:
                                nc.vector.scalar_tensor_tensor(
                                    out=dst[:, clo:chi], in0=a[:, clo:chi],
                                    scalar=1.0, in1=b[:, clo:chi],
                                    op0=MULT, op1=MULT, accum_out=mc,
                                )
                        continue
                    if eng == "A":
                        assert a_name == b_name, (s, "ACT needs a square")
                        nc.scalar.activation(
                            out=dst[:, lo:hi], in_=a[:, lo:hi], func=SQUARE,
                            accum_out=mcol,
                        )
                    elif eng == "D":
                        nc.vector.scalar_tensor_tensor(
                            out=dst[:, lo:hi], in0=a[:, lo:hi], scalar=1.0,
                            in1=b[:, lo:hi], op0=MULT, op1=MULT,
                            accum_out=mcol,
                        )
                    elif eng == "G":
                        nc.gpsimd.scalar_tensor_tensor(
                            out=dst[:, lo:hi], in0=a[:, lo:hi], scalar=1.0,
                            in1=b[:, lo:hi], op0=MULT, op1=MULT,
                            accum_out=mcol,
                        )
                    else:
                        raise ValueError(eng)

                # Transpose moments: [128, K*] -> [K*, 128] PSUM, copy to SBUF
                if mt_batch > 1:
                    if it % mt_batch != mt_batch - 1:
                        continue_tail = True
                    mt_rows = None
                    if it % mt_batch == mt_batch - 1:
                        mt_ps = pt.tile([mt_batch * 32, P], F32, tag="mtp")
                        nc.tensor.transpose(mt_ps[:, :], m_pair[:, :], ident[:, :])
                        mt_all = mtsb.tile([mt_batch * 32, P], F32, tag="mta")
                        if cfg["mt"] == "D":
                            nc.vector.tensor_copy(mt_all[:, :], mt_ps[:, :])
                        else:
                            nc.scalar.copy(mt_all[:, :], mt_ps[:, :])
                    else:
                        continue
                else:
                    mt_ps = pt.tile([mcols, P], F32)
                    nc.tensor.transpose(mt_ps[:, :], m_t[:, :], ident[:, :])
                    mt_sb = mtsb.tile([mcols, P], F32)
                    if cfg["mt"] == "D":
                        r = nc.vector.tensor_copy(mt_sb[:, :], mt_ps[:, :])
                    else:
                        r = nc.scalar.copy(mt_sb[:, :], mt_ps[:, :])
                    if cfg.get("mt_prio_bump", 0):
                        r.ins.bass_priority += cfg["mt_prio_bump"]

                # out[128, 1024] = MT.T @ GT  (contraction K)
                sub_tiles = ([it] if cfg.get("mt_batch", 1) == 1 else
                             list(range(it - cfg["mt_batch"] + 1, it + 1)))
                for sit in sub_tiles:
                    if cfg.get("mt_batch", 1) > 1:
                        sidx = sit - (it - cfg["mt_batch"] + 1)
                        mt_sb = mt_all[sidx * 32:sidx * 32 + mcols, :]
                        gt_use = gt_sb[sidx * 32:sidx * 32 + mcols, :]
                    else:
                        gt_use = gt_sb[:, :]
                    _emit_out(nc, cfg, oc_pair, pout, ostage, mt_sb, gt_use, out, sit)

    nc.finalize()
    return nc


_NC_CACHE: dict[tuple, bass.Bass] = {}


def _host_gt(coeffs, cfg=CFG):
    C = _cheb_monomial_matrix()
    G = (coeffs.astype(np.float64) @ C).astype(np.float32)  # [OUT_F, W]
    rows = _moment_rows(cfg)
    GT = np.ascontiguousarray(G.T[rows, :])  # [K, OUT_F]
    return GT


def _run(x, coeffs, input_scale, cfg=CFG, **spmd_kwargs):
    x = np.ascontiguousarray(np.asarray(x, dtype=np.float32))
    coeffs = np.asarray(coeffs, dtype=np.float32)
    scale = float(np.clip(np.asarray(input_scale, dtype=np.float32), 0.1, 2.0).reshape(-1)[0])

    GT = _host_gt(coeffs, cfg)

    key = (scale, str(cfg))
    nc = _NC_CACHE.get(key)
    if nc is None:
        nc = _build_nc(scale, cfg)
        _NC_CACHE[key] = nc

    in_maps = [
        {"x": np.ascontiguousarray(x[c * ROWS_PER_CORE:(c + 1) * ROWS_PER_CORE]),
         "gt": GT}
        for c in range(N_CORES)
    ]
    res = run_bass_kernel_spmd(nc, in_maps, core_ids=list(range(N_CORES)), **spmd_kwargs)
    out = np.concatenate([res.results[c]["out"] for c in range(N_CORES)], axis=0)
    return out.astype(np.float32), res


def kernel(x, coeffs, input_scale):
    out, _ = _run(x, coeffs, input_scale)
    return out


if __name__ == "__main__":
    rng = np.random.default_rng(0)
    x = rng.standard_normal((BATCH, IN_F), dtype=np.float32)
    coeffs = (rng.standard_normal((OUT_F, W)) * 0.1).astype(np.float32)
    s = np.ones((1,), np.float32)
    out = kernel(x=x, coeffs=coeffs, input_scale=s)
    print(out.shape, out.dtype)



# revision 17
# speedup vs baseline: 1.1922x; 1.1922x over previous
"""Trainium2 Bass kernel for ChebyshevActivation.

Math:
    scale = clip(input_scale, 0.1, 2.0)
    t = tanh(x * scale)                        # t in (-1, 1)
    out[b, o] = sum_w coeffs[o, w] * sum_i T_w(t[b, i])

Since |t| < 1, all T_n(t) lie in [-1, 1] and the reference's clip(+-100) is
dead code.  Work in the monomial basis: with power sums M_j[b] = sum_i t^j
(M_0 = IN_F exactly) and G = coeffs @ C (C the Chebyshev->monomial matrix),
out = M @ G^T.

Engine cost model (per [128, 2048] f16 pass, TimelineSim-validated):
  ACT activation (tanh/square, fused accum): 2079 ns
  DVE tensor_tensor f16 mult (2x_1p):        1127 ns
  DVE tensor_scalar f16 + accum (4x_2p):      594 ns
  Pool tensor_tensor f16 mult (0.42 eff):    4253 ns
So: products run on DVE at 2x with separate 4x tensor_scalar reductions
(1721/moment) instead of the 1x fused scalar_tensor_tensor (2194/moment);
ACT takes tanh + a balanced share of the squares with fused accum_out; Pool
(which cannot touch PSUM or run TensorScalarPtr, but does accept f16
TensorTensor) absorbs the t4 product and part of j8.  The final matmul runs
in float32r (1 cycle/row vs 4 for f32); moment columns can be split across
engines column-wise, with the host duplicating G rows to re-merge partial
sums (the final PE matmul re-merges them).

Per-core layout: data-parallel over batch, 8 cores x 1024 rows,
8 row-tiles of [128, 2048] per core.
"""

import numpy as np

import concourse.bass as bass
import concourse.bacc as bacc
import concourse.mybir as mybir
import concourse.tile as tile
from concourse import masks
from concourse.bass_utils import run_bass_kernel_spmd

# This environment's walrus build rejects raw client-encoded ISA instructions
# ("ISA wrong length" for the 64-byte EVENT_SEMAPHORE_RANGE_CLEAR emitted by
# the TileContext exit barrier).  Replace the range-clear with per-semaphore
# EventSemaphore writes (update_mode=sem-wr-imm, value 0), which this walrus
# accepts, so re-executing the loaded NEFF still sees cleared semaphores.
def _sem_clear_via_events(self, sem_range):
    engines = list(self.bass.engines.values())
    inst = None
    for i, s in enumerate(sem_range):
        eng = engines[i % len(engines)]
        inst = mybir.InstEventSemaphore(
            name=self.bass.get_next_instruction_name(),
            ins=[], outs=[],
            sync_info=mybir.SyncInfo(
                on_wait=[],
                on_update=[mybir.SyncUpdate(
                    sync_type="semaphore", id=s,
                    update_mode="sem-wr-imm", update_value=0,
                )],
            ),
        )
        eng.add_instruction(inst)
    return inst


bass.BassGpSimd.sem_clear = _sem_clear_via_events

N_CORES = 8
BATCH = 8192
IN_F = 2048
OUT_F = 1024
DEG = 8
W = DEG + 1  # 9 moments
ROWS_PER_CORE = BATCH // N_CORES  # 1024
P = 128
NTILES = ROWS_PER_CORE // P  # 8

F32 = mybir.dt.float32
F32R = mybir.dt.float32r
F16 = mybir.dt.float16
MULT = mybir.AluOpType.mult
ADD = mybir.AluOpType.add
SQUARE = mybir.ActivationFunctionType.Square
TANH = mybir.ActivationFunctionType.Tanh

# Stream definitions: name -> (in0, in1, is_value).  in0 == in1 => square
# (ACT-eligible).  Values t2/t3/t4 are real channels; v5/j6/v7/j8 are junk
# (only their accumulated moment matters).
STREAMS = {
    "t2": ("t1", "t1", True),
    "t3": ("t2", "t1", True),
    "t4": ("t2", "t2", True),
    "v5": ("t4", "t1", False),
    "j6": ("t3", "t3", False),
    "v7": ("t4", "t3", False),
    "j8": ("t4", "t4", False),
}
STREAM_MOMENT = {"t2": 2, "t3": 3, "t4": 4, "v5": 5, "j6": 6, "v7": 7, "j8": 8}

# Engine assignment: stream -> list of (engine, fraction) summing to 1.
# "A" = ACT square (fused accum; squares only), "D" = DVE tt + DVE ts,
# "G" = Pool tt + DVE ts.
CFG = {
    "t2": [("A", 1.0)],
    "t3": [("D", 1.0)],
    "t4": [("D", 1.0)],
    "v5": [("G", 0.5), ("D", 0.5)],
    "j6": [("A", 1.0)],
    "v7": [("D", 1.0)],
    "j8": [("G", 1.0)],
    "oc_act": 1.0,   # fraction of the PSUM->SBUF output copy done on ACT
    "mt": "D",         # moment-transpose PSUM->SBUF copy engine
    "xin_bufs": 4,
    "chan_bufs": 3,
    "t1_bufs": 3,
    "ostage_bufs": 3,
    "tail_fast": True,
    "tail_chunks": 2,
    # tile 7: j8 moves Pool->ACT so Pool latency isn't in the tail chain
    "last_no_G": True,
    "last_G_eng": "A",
    # tile 0: chunk tanh into quarters (extra partial-moment columns,
    # re-merged by duplicated G rows) so downstream engines start sooner
    "warm": True,
    "warm_set": (),
    "tanh_chunks": 4,
    # deprioritize DVE reductions of Pool-produced values so they don't
    # head-of-line-block the next tile's DVE products
    "gts_prio": 16,
}


def _cheb_monomial_matrix(deg=DEG):
    C = np.zeros((deg + 1, deg + 1), dtype=np.float64)
    C[0, 0] = 1.0
    if deg >= 1:
        C[1, 1] = 1.0
    for n in range(2, deg + 1):
        C[n, 1:] = 2.0 * C[n - 1, :-1]
        C[n, :] -= C[n - 2, :]
    return C


def _pieces(cfg):
    """Deterministic piece list: (stream, engine, col_lo, col_hi)."""
    out = []
    for s in STREAMS:
        cols = 0
        parts = cfg[s]
        for idx, (eng, frac) in enumerate(parts):
            if idx == len(parts) - 1:
                hi = IN_F
            else:
                hi = cols + int(round(IN_F * frac / 128.0)) * 128
                hi = min(hi, IN_F)
            if hi > cols:
                out.append((s, eng, cols, hi))
            cols = hi
    return out


def _moment_rows(cfg):
    """Row j of GT corresponds to these monomial moments: [M0, M1, *pieces,
    then warm-duplicate rows for tile 0's chunked streams (tanh + each
    full-width piece)]."""
    ps = _pieces(cfg)
    rows = [0, 1]
    for s, _eng, _lo, _hi in ps:
        rows.append(STREAM_MOMENT[s])
    if cfg.get("warm"):
        wset = cfg.get("warm_set", ("t2",))
        rows += [1, 1, 1]  # tanh quarters 2-4
        for s, _eng, lo, hi in ps:
            if lo == 0 and hi == IN_F and s in wset:
                rows.append(STREAM_MOMENT[s])
    return rows


def _build_nc(scale: float, cfg=CFG) -> bass.Bass:
    pieces = _pieces(cfg)
    warm = bool(cfg.get("warm"))
    wset = cfg.get("warm_set", ("t2",))
    full = [i for i, (s_, e_, lo, hi) in enumerate(pieces)
            if lo == 0 and hi == IN_F and s_ in wset]
    nwarm = (3 + len(full)) if warm else 0  # tanh quarters + full-width pieces
    K = 2 + len(pieces) + nwarm
    assert K <= 30
    mcols = K
    # warm col for piece i (tile 0 second-half accum): wcol_of[i]
    wcol_of = {pi: 2 + len(pieces) + 3 + j for j, pi in enumerate(full)}

    nc = bacc.Bacc("TRN2")
    x = nc.dram_tensor("x", [ROWS_PER_CORE, IN_F], F32, kind="ExternalInput")
    gt = nc.dram_tensor("gt", [K, OUT_F], F32, kind="ExternalInput")
    out = nc.dram_tensor("out", [ROWS_PER_CORE, OUT_F], F32, kind="ExternalOutput")

    with tile.TileContext(nc) as tc:
        with (
            tc.tile_pool(name="singles", bufs=1) as singles,
            tc.tile_pool(name="xin", bufs=cfg["xin_bufs"]) as xin,
            tc.tile_pool(name="chan", bufs=cfg["chan_bufs"]) as chan,
            tc.tile_pool(name="chan1", bufs=cfg.get("t1_bufs", cfg["chan_bufs"])) as chan1,
            tc.tile_pool(name="junk", bufs=2) as junkp,
            tc.tile_pool(name="mpool", bufs=4) as mpool,
            tc.tile_pool(name="mtsb", bufs=4) as mtsb,
            tc.tile_pool(name="ostage", bufs=cfg["ostage_bufs"]) as ostage,
            tc.tile_pool(name="pt", bufs=2, space="PSUM") as pt,
            tc.tile_pool(name="pout", bufs=2, space="PSUM") as pout,
        ):
            gt_f = singles.tile([K, OUT_F], F32)
            nc.sync.dma_start(out=gt_f[:, :], in_=gt[:, :])
            gt_sb = singles.tile([K, OUT_F], F32R)
            nc.vector.tensor_copy(gt_sb[:, :], gt_f[:, :])
            ident = singles.tile([P, P], F32)
            masks.make_identity(nc, ident[:, :])

            # Tiny dummy activation on an early-memset tile: the Bacc-inserted
            # ACT_TABLE_LOAD attaches before the first Activation instruction
            # and would otherwise inherit the first tanh's DMA waits, pushing
            # the 1.3us table load into the critical ramp.
            dum = singles.tile([P, 1], F16)
            nc.gpsimd.memset(dum[:, :], 0.0)
            nc.scalar.activation(out=dum[:, :], in_=dum[:, :], func=TANH)

            for it in range(NTILES):
                j_dve = junkp.tile([P, IN_F], F16, tag="jd")
                j_act = junkp.tile([P, IN_F], F16, tag="ja")
                j_gps = junkp.tile([P, IN_F], F16, tag="jg")
                j_ts = junkp.tile([P, IN_F], F16, tag="jt")
                JUNK = {"A": j_act, "D": j_dve, "G": j_gps}
                chunked = warm and it == 0
                last = it == NTILES - 1
                H = IN_F // 2
                x_t = xin.tile([P, IN_F], F32)
                if chunked:
                    NQ = cfg.get("dma0_chunks", 4)
                    Q = IN_F // NQ
                    for q in range(NQ):
                        nc.sync.dma_start(out=x_t[:, q * Q:(q + 1) * Q],
                                          in_=x[it * P:(it + 1) * P, q * Q:(q + 1) * Q])
                else:
                    nc.sync.dma_start(out=x_t[:, :], in_=x[it * P:(it + 1) * P, :])

                m_t = mpool.tile([P, mcols], F32)
                nc.gpsimd.memset(m_t[:, 0:1], float(IN_F))
                if warm and not chunked:
                    # zero the warm duplicate cols on non-chunked tiles
                    nc.gpsimd.memset(m_t[:, K - nwarm:K], 0.0)

                t1 = chan1.tile([P, IN_F], F16, tag="t1")
                t2 = chan.tile([P, IN_F], F16, tag="t2")
                t3 = chan.tile([P, IN_F], F16, tag="t3")
                t4 = chan.tile([P, IN_F], F16, tag="t4")
                VALS = {"t1": t1, "t2": t2, "t3": t3, "t4": t4}

                # t1 = tanh(scale * x), accum -> M1 (col 1; chunk b -> warm col)
                if chunked:
                    wc = 2 + len(pieces)
                    NCH = cfg.get("tanh_chunks", 2)
                    Q = IN_F // NCH
                    for q in range(NCH):
                        nc.scalar.activation(
                            out=t1[:, q * Q:(q + 1) * Q],
                            in_=x_t[:, q * Q:(q + 1) * Q], func=TANH,
                            scale=scale,
                            accum_out=(m_t[:, 1:2] if q == 0
                                       else m_t[:, wc + q - 1:wc + q]),
                        )
                else:
                    nc.scalar.activation(
                        out=t1[:, :], in_=x_t[:, :], func=TANH,
                        scale=scale, accum_out=m_t[:, 1:2],
                    )

                def emit_piece(s, eng, lo, hi, mcol):
                    a_name, b_name, is_val = STREAMS[s]
                    a = VALS[a_name]
                    b = VALS[b_name]
                    dst = VALS[s] if is_val else JUNK[eng]
                    if eng == "A":
                        assert a_name == b_name, (s, "ACT needs a square")
                        nc.scalar.activation(
                            out=dst[:, lo:hi], in_=a[:, lo:hi], func=SQUARE,
                            accum_out=mcol,
                        )
                    elif eng == "D":
                        nc.vector.tensor_tensor(
                            dst[:, lo:hi], a[:, lo:hi], b[:, lo:hi], MULT)
                        if mcol is None:
                            return
                        nc.vector.tensor_scalar(
                            j_ts[:, lo:hi], dst[:, lo:hi], 1.0, 0.0, MULT, ADD,
                            accum_out=mcol)
                    elif eng == "G":
                        nc.gpsimd.tensor_tensor(
                            dst[:, lo:hi], a[:, lo:hi], b[:, lo:hi], MULT)
                        if mcol is None:
                            return
                        r = nc.vector.tensor_scalar(
                            j_ts[:, lo:hi], dst[:, lo:hi], 1.0, 0.0, MULT, ADD,
                            accum_out=mcol)
                        if cfg.get("gts_prio", 0):
                            r.ins.bass_priority += cfg["gts_prio"]
                    else:
                        raise ValueError(eng)

                merged = cfg.get("merge_ts", ())
                first_col = {}
                for pidx, (s, eng, lo, hi) in enumerate(pieces):
                    if eng == "G" and last and cfg.get("last_no_G"):
                        a_nm, b_nm, _iv = STREAMS[s]
                        eng = cfg.get("last_G_eng", "D")
                        if eng == "A" and a_nm != b_nm:
                            eng = "D"
                    mcol = m_t[:, 2 + pidx:3 + pidx]
                    if s in merged and not STREAMS[s][2]:
                        # all pieces of this split junk stream write j_gps;
                        # one full-width ts accumulates into the first col
                        # (after the last piece); other cols are zeroed
                        JUNK_s = JUNK
                        JUNK = {k: j_gps for k in JUNK}
                        emit_piece(s, eng, lo, hi, None)
                        JUNK = JUNK_s
                        if s not in first_col:
                            first_col[s] = mcol
                            nc.gpsimd.memset(mcol, 0.0)  # placeholder order
                        else:
                            nc.gpsimd.memset(mcol, 0.0)
                        if hi == IN_F:
                            r = nc.vector.tensor_scalar(
                                j_ts[:, :], j_gps[:, :], 1.0, 0.0, MULT, ADD,
                                accum_out=first_col[s])
                            if cfg.get("gts_prio", 0):
                                r.ins.bass_priority += cfg["gts_prio"]
                        continue
                    if chunked and pidx in wcol_of:
                        wc = wcol_of[pidx]
                        emit_piece(s, eng, 0, H, mcol)
                        emit_piece(s, eng, H, IN_F, m_t[:, wc:wc + 1])
                    else:
                        emit_piece(s, eng, lo, hi, mcol)

                # Transpose moments: [128, K] -> [K, 128] PSUM, copy to SBUF
                mt_ps = pt.tile([mcols, P], F32)
                nc.tensor.transpose(mt_ps[:, :], m_t[:, :], ident[:, :])
                mt_sb = mtsb.tile([mcols, P], F32R)
                if cfg["mt"] == "D":
                    nc.vector.tensor_copy(mt_sb[:, :], mt_ps[:, :])
                else:
                    nc.scalar.copy(mt_sb[:, :], mt_ps[:, :])

                # out[128, 1024] = MT.T @ GT  (contraction K), f32r
                o_ps = pout.tile([P, OUT_F], F32)
                for h in range(2):
                    nc.tensor.matmul(
                        o_ps[:, h * 512:(h + 1) * 512],
                        lhsT=mt_sb[:, :],
                        rhs=gt_sb[:, h * 512:(h + 1) * 512],
                        start=True, stop=True,
                    )
                o_sb = ostage.tile([P, OUT_F], F32)
                if cfg.get("tail_fast") and it == NTILES - 1:
                    # chunk the last tile's output: copy (alternating
                    # ACT/DVE) + DMA per chunk, pipelined
                    NTC = cfg.get("tail_chunks", 2)
                    QO = OUT_F // NTC
                    for qo in range(NTC):
                        sl = slice(qo * QO, (qo + 1) * QO)
                        if qo % 2 == 0:
                            nc.scalar.copy(o_sb[:, sl], o_ps[:, sl])
                        else:
                            nc.vector.tensor_copy(o_sb[:, sl], o_ps[:, sl])
                        nc.sync.dma_start(out=out[it * P:(it + 1) * P, sl],
                                          in_=o_sb[:, sl])
                    continue
                ca = int(round(OUT_F * cfg["oc_act"] / 128.0)) * 128
                ca = max(0, min(OUT_F, ca))
                ocb = cfg.get("oc_prio", 0)
                if ca > 0:
                    r = nc.scalar.copy(o_sb[:, 0:ca], o_ps[:, 0:ca])
                    if ocb:
                        r.ins.bass_priority += ocb
                if ca < OUT_F:
                    r = nc.vector.tensor_copy(o_sb[:, ca:OUT_F], o_ps[:, ca:OUT_F])
                    if ocb:
                        r.ins.bass_priority += ocb
                nc.sync.dma_start(out=out[it * P:(it + 1) * P, :], in_=o_sb[:, :])

    nc.finalize()
    return nc


_NC_CACHE: dict[tuple, bass.Bass] = {}


def _host_gt(coeffs, cfg=CFG):
    C = _cheb_monomial_matrix()
    G = (coeffs.astype(np.float64) @ C).astype(np.float32)  # [OUT_F, W]
    rows = _moment_rows(cfg)
    GT = np.ascontiguousarray(G.T[rows, :])  # [K, OUT_F]
    return GT


def _run(x, coeffs, input_scale, cfg=CFG, **spmd_kwargs):
    x = np.ascontiguousarray(np.asarray(x, dtype=np.float32))
    coeffs = np.asarray(coeffs, dtype=np.float32)
    scale = float(np.clip(np.asarray(input_scale, dtype=np.float32), 0.1, 2.0).reshape(-1)[0])

    GT = _host_gt(coeffs, cfg)

    key = (scale, str(cfg))
    nc = _NC_CACHE.get(key)
    if nc is None:
        nc = _build_nc(scale, cfg)
        _NC_CACHE[key] = nc

    in_maps = [
        {"x": np.ascontiguousarray(x[c * ROWS_PER_CORE:(c + 1) * ROWS_PER_CORE]),
         "gt": GT}
        for c in range(N_CORES)
    ]
    res = run_bass_kernel_spmd(nc, in_maps, core_ids=list(range(N_CORES)), **spmd_kwargs)
    out = np.concatenate([res.results[c]["out"] for c in range(N_CORES)], axis=0)
    return out.astype(np.float32), res


def kernel(x, coeffs, input_scale):
    out, _ = _run(x, coeffs, input_scale)
    return out


if __name__ == "__main__":
    rng = np.random.default_rng(0)
    x = rng.standard_normal((BATCH, IN_F), dtype=np.float32)
    coeffs = (rng.standard_normal((OUT_F, W)) * 0.1).astype(np.float32)
    s = np.ones((1,), np.float32)
    out = kernel(x=x, coeffs=coeffs, input_scale=s)
    print(out.shape, out.dtype)


# revision 22
# speedup vs baseline: 1.2064x; 1.0120x over previous
"""Trainium2 Bass kernel for ChebyshevActivation.

Math:
    scale = clip(input_scale, 0.1, 2.0)
    t = tanh(x * scale)                        # t in (-1, 1)
    out[b, o] = sum_w coeffs[o, w] * sum_i T_w(t[b, i])

Since |t| < 1, all T_n(t) lie in [-1, 1] and the reference's clip(+-100) is
dead code.  Work in the monomial basis: with power sums M_j[b] = sum_i t^j
(M_0 = IN_F exactly) and G = coeffs @ C (C the Chebyshev->monomial matrix),
out = M @ G^T.

Engine cost model (per [128, 2048] f16 pass, TimelineSim-validated):
  ACT activation (tanh/square, fused accum): 2079 ns
  DVE tensor_tensor f16 mult (2x_1p):        1127 ns
  DVE tensor_scalar f16 + accum (4x_2p):      594 ns
  Pool tensor_tensor f16 mult (0.42 eff):    4253 ns
So: products run on DVE at 2x with separate 4x tensor_scalar reductions
(1721/moment) instead of the 1x fused scalar_tensor_tensor (2194/moment);
ACT takes tanh + a balanced share of the squares with fused accum_out; Pool
(which cannot touch PSUM or run TensorScalarPtr, but does accept f16
TensorTensor) absorbs the t4 product and part of j8.  The final matmul runs
in float32r (1 cycle/row vs 4 for f32); moment columns can be split across
engines column-wise, with the host duplicating G rows to re-merge partial
sums (the final PE matmul re-merges them).

Per-core layout: data-parallel over batch, 8 cores x 1024 rows,
8 row-tiles of [128, 2048] per core.
"""

import numpy as np

import concourse.bass as bass
import concourse.bacc as bacc
import concourse.mybir as mybir
import concourse.tile as tile
from concourse import masks
from concourse.bass_utils import run_bass_kernel_spmd

# This environment's walrus build rejects raw client-encoded ISA instructions
# ("ISA wrong length" for the 64-byte EVENT_SEMAPHORE_RANGE_CLEAR emitted by
# the TileContext exit barrier).  Replace the range-clear with per-semaphore
# EventSemaphore writes (update_mode=sem-wr-imm, value 0), which this walrus
# accepts, so re-executing the loaded NEFF still sees cleared semaphores.
def _sem_clear_via_events(self, sem_range):
    engines = list(self.bass.engines.values())
    inst = None
    for i, s in enumerate(sem_range):
        eng = engines[i % len(engines)]
        inst = mybir.InstEventSemaphore(
            name=self.bass.get_next_instruction_name(),
            ins=[], outs=[],
            sync_info=mybir.SyncInfo(
                on_wait=[],
                on_update=[mybir.SyncUpdate(
                    sync_type="semaphore", id=s,
                    update_mode="sem-wr-imm", update_value=0,
                )],
            ),
        )
        eng.add_instruction(inst)
    return inst


bass.BassGpSimd.sem_clear = _sem_clear_via_events

N_CORES = 8
BATCH = 8192
IN_F = 2048
OUT_F = 1024
DEG = 8
W = DEG + 1  # 9 moments
ROWS_PER_CORE = BATCH // N_CORES  # 1024
P = 128
NTILES = ROWS_PER_CORE // P  # 8

F32 = mybir.dt.float32
F32R = mybir.dt.float32r
F16 = mybir.dt.float16
MULT = mybir.AluOpType.mult
ADD = mybir.AluOpType.add
SQUARE = mybir.ActivationFunctionType.Square
TANH = mybir.ActivationFunctionType.Tanh

# Stream definitions: name -> (in0, in1, is_value).  in0 == in1 => square
# (ACT-eligible).  Values t2/t3/t4 are real channels; v5/j6/v7/j8 are junk
# (only their accumulated moment matters).
STREAMS = {
    "t2": ("t1", "t1", True),
    "t3": ("t2", "t1", True),
    "t4": ("t2", "t2", True),
    "v5": ("t4", "t1", False),
    "j6": ("t3", "t3", False),
    "v7": ("t4", "t3", False),
    "j8": ("t4", "t4", False),
}
STREAM_MOMENT = {"t2": 2, "t3": 3, "t4": 4, "v5": 5, "j6": 6, "v7": 7, "j8": 8}

# Engine assignment: stream -> list of (engine, fraction) summing to 1.
# "A" = ACT square (fused accum; squares only), "D" = DVE tt + DVE ts,
# "G" = Pool tt + DVE ts.
CFG = {
    "t2": [("A", 1.0)],
    "t3": [("D", 1.0)],
    "t4": [("D", 1.0)],
    "v5": [("G", 0.5), ("D", 0.5)],
    "j6": [("A", 1.0)],
    "v7": [("D", 1.0)],
    "j8": [("G", 1.0)],
    "oc_act": 1.0,   # fraction of the PSUM->SBUF output copy done on ACT
    "mt": "D",         # moment-transpose PSUM->SBUF copy engine
    "xin_bufs": 4,
    "chan_bufs": 3,
    "t1_bufs": 3,
    "ostage_bufs": 3,
    "tail_fast": True,
    "tail_chunks": 2,
    # tile 7: j8 moves Pool->ACT so Pool latency isn't in the tail chain
    "last_no_G": True,
    "last_G_eng": "A",
    # tile 0: chunk tanh into quarters (extra partial-moment columns,
    # re-merged by duplicated G rows) so downstream engines start sooner
    "warm": True,
    "warm_set": (),
    "tanh_chunks": 4,
    # deprioritize DVE reductions of Pool-produced values so they don't
    # head-of-line-block the next tile's DVE products
    "gts_prio": 16,
    # tile 7: t4 as a fused ACT square so the DVE tail chain is shorter
    "last_t4_A": True,
}


def _cheb_monomial_matrix(deg=DEG):
    C = np.zeros((deg + 1, deg + 1), dtype=np.float64)
    C[0, 0] = 1.0
    if deg >= 1:
        C[1, 1] = 1.0
    for n in range(2, deg + 1):
        C[n, 1:] = 2.0 * C[n - 1, :-1]
        C[n, :] -= C[n - 2, :]
    return C


def _pieces(cfg):
    """Deterministic piece list: (stream, engine, col_lo, col_hi)."""
    out = []
    for s in STREAMS:
        cols = 0
        parts = cfg[s]
        for idx, (eng, frac) in enumerate(parts):
            if idx == len(parts) - 1:
                hi = IN_F
            else:
                hi = cols + int(round(IN_F * frac / 128.0)) * 128
                hi = min(hi, IN_F)
            if hi > cols:
                out.append((s, eng, cols, hi))
            cols = hi
    return out


def _moment_rows(cfg):
    """Row j of GT corresponds to these monomial moments: [M0, M1, *pieces,
    then warm-duplicate rows for tile 0's chunked streams (tanh + each
    full-width piece)].  With m0_host, M0 is folded in on the host."""
    ps = _pieces(cfg)
    rows = ([1] if cfg.get("m0_host") else [0, 1])
    for s, _eng, _lo, _hi in ps:
        rows.append(STREAM_MOMENT[s])
    if cfg.get("warm"):
        wset = cfg.get("warm_set", ("t2",))
        rows += [1, 1, 1]  # tanh quarters 2-4
        for s, _eng, lo, hi in ps:
            if lo == 0 and hi == IN_F and s in wset:
                rows.append(STREAM_MOMENT[s])
    return rows


def _build_nc(scale: float, cfg=CFG) -> bass.Bass:
    pieces = _pieces(cfg)
    warm = bool(cfg.get("warm"))
    wset = cfg.get("warm_set", ("t2",))
    m0h = bool(cfg.get("m0_host"))
    nfixed = 1 if m0h else 2  # [M1] or [M0, M1]
    full = [i for i, (s_, e_, lo, hi) in enumerate(pieces)
            if lo == 0 and hi == IN_F and s_ in wset]
    nwarm = (3 + len(full)) if warm else 0  # tanh quarters + full-width pieces
    K = nfixed + len(pieces) + nwarm
    assert K <= 30
    mcols = K
    # warm col for piece i (tile 0 second-half accum): wcol_of[i]
    wcol_of = {pi: nfixed + len(pieces) + 3 + j for j, pi in enumerate(full)}

    nc = bacc.Bacc("TRN2")
    XDT = F16 if cfg.get("x_f16") else F32
    x = nc.dram_tensor("x", [ROWS_PER_CORE, IN_F], XDT, kind="ExternalInput")
    gt = nc.dram_tensor("gt", [K, OUT_F], F32, kind="ExternalInput")
    out = nc.dram_tensor("out", [ROWS_PER_CORE, OUT_F], F32, kind="ExternalOutput")

    with tile.TileContext(nc) as tc:
        with (
            tc.tile_pool(name="singles", bufs=1) as singles,
            tc.tile_pool(name="xin", bufs=cfg["xin_bufs"]) as xin,
            tc.tile_pool(name="chan", bufs=cfg["chan_bufs"]) as chan,
            tc.tile_pool(name="chan1", bufs=cfg.get("t1_bufs", cfg["chan_bufs"])) as chan1,
            tc.tile_pool(name="junk", bufs=2) as junkp,
            tc.tile_pool(name="mpool", bufs=4) as mpool,
            tc.tile_pool(name="mtsb", bufs=4) as mtsb,
            tc.tile_pool(name="ostage", bufs=cfg["ostage_bufs"]) as ostage,
            tc.tile_pool(name="pt", bufs=2, space="PSUM") as pt,
            tc.tile_pool(name="pout", bufs=2, space="PSUM") as pout,
        ):
            gt_f = singles.tile([K, OUT_F], F32)
            MMDT = F32 if cfg.get("mm_f32") else F32R
            gt_sb = singles.tile([K, OUT_F], MMDT)
            if not cfg.get("gt_defer"):
                nc.sync.dma_start(out=gt_f[:, :], in_=gt[:, :])
                nc.vector.tensor_copy(gt_sb[:, :], gt_f[:, :])
            ident = singles.tile([P, P], F32)
            masks.make_identity(nc, ident[:, :])

            # Tiny dummy activation on an early-memset tile: the Bacc-inserted
            # ACT_TABLE_LOAD attaches before the first Activation instruction
            # and would otherwise inherit the first tanh's DMA waits, pushing
            # the 1.3us table load into the critical ramp.
            dum = singles.tile([P, 1], F16)
            nc.gpsimd.memset(dum[:, :], 0.0)
            nc.scalar.activation(out=dum[:, :], in_=dum[:, :], func=TANH)

            for it in range(NTILES):
                j_dve = junkp.tile([P, IN_F], F16, tag="jd")
                j_act = junkp.tile([P, IN_F], F16, tag="ja")
                j_gps = junkp.tile([P, IN_F], F16, tag="jg")
                j_ts = junkp.tile([P, IN_F], F16, tag="jt")
                JUNK = {"A": j_act, "D": j_dve, "G": j_gps}
                chunked = warm and it == 0
                last = it >= NTILES - cfg.get("last_k", 1)
                H = IN_F // 2
                x_t = xin.tile([P, IN_F], XDT)
                if chunked:
                    NQ = cfg.get("dma0_chunks", 4)
                    Q = IN_F // NQ
                    for q in range(NQ):
                        nc.sync.dma_start(out=x_t[:, q * Q:(q + 1) * Q],
                                          in_=x[it * P:(it + 1) * P, q * Q:(q + 1) * Q])
                else:
                    nc.sync.dma_start(out=x_t[:, :], in_=x[it * P:(it + 1) * P, :])
                if it == 0 and cfg.get("gt_defer"):
                    nc.sync.dma_start(out=gt_f[:, :], in_=gt[:, :])
                    nc.vector.tensor_copy(gt_sb[:, :], gt_f[:, :])

                m_t = mpool.tile([P, mcols], F32)
                if not m0h:
                    nc.gpsimd.memset(m_t[:, 0:1], float(IN_F))
                if warm and not chunked:
                    # zero the warm duplicate cols on non-chunked tiles
                    nc.gpsimd.memset(m_t[:, K - nwarm:K], 0.0)

                t1 = chan1.tile([P, IN_F], F16, tag="t1")
                t2 = chan.tile([P, IN_F], F16, tag="t2")
                t3 = chan.tile([P, IN_F], F16, tag="t3")
                t4 = chan.tile([P, IN_F], F16, tag="t4")
                VALS = {"t1": t1, "t2": t2, "t3": t3, "t4": t4}

                # t1 = tanh(scale * x), accum -> M1 (col 1; chunk b -> warm col)
                if chunked:
                    wc = nfixed + len(pieces)
                    NCH = cfg.get("tanh_chunks", 2)
                    Q = IN_F // NCH
                    for q in range(NCH):
                        nc.scalar.activation(
                            out=t1[:, q * Q:(q + 1) * Q],
                            in_=x_t[:, q * Q:(q + 1) * Q], func=TANH,
                            scale=scale,
                            accum_out=(m_t[:, nfixed - 1:nfixed] if q == 0
                                       else m_t[:, wc + q - 1:wc + q]),
                        )
                else:
                    nc.scalar.activation(
                        out=t1[:, :], in_=x_t[:, :], func=TANH,
                        scale=scale, accum_out=m_t[:, nfixed - 1:nfixed],
                    )

                def emit_piece(s, eng, lo, hi, mcol):
                    a_name, b_name, is_val = STREAMS[s]
                    a = VALS[a_name]
                    b = VALS[b_name]
                    dst = VALS[s] if is_val else JUNK[eng]
                    if eng == "A":
                        assert a_name == b_name, (s, "ACT needs a square")
                        nc.scalar.activation(
                            out=dst[:, lo:hi], in_=a[:, lo:hi], func=SQUARE,
                            accum_out=mcol,
                        )
                    elif eng == "D":
                        nc.vector.tensor_tensor(
                            dst[:, lo:hi], a[:, lo:hi], b[:, lo:hi], MULT)
                        if mcol is None:
                            return
                        nc.vector.tensor_scalar(
                            j_ts[:, lo:hi], dst[:, lo:hi], 1.0, 0.0, MULT, ADD,
                            accum_out=mcol)
                    elif eng == "G":
                        nc.gpsimd.tensor_tensor(
                            dst[:, lo:hi], a[:, lo:hi], b[:, lo:hi], MULT)
                        if mcol is None:
                            return
                        r = nc.vector.tensor_scalar(
                            j_ts[:, lo:hi], dst[:, lo:hi], 1.0, 0.0, MULT, ADD,
                            accum_out=mcol)
                        if cfg.get("gts_prio", 0):
                            r.ins.bass_priority += cfg["gts_prio"]
                    else:
                        raise ValueError(eng)

                merged = cfg.get("merge_ts", ())
                first_col = {}
                for pidx, (s, eng, lo, hi) in enumerate(pieces):
                    if eng == "G" and last and cfg.get("last_no_G"):
                        a_nm, b_nm, _iv = STREAMS[s]
                        eng = cfg.get("last_G_eng", "D")
                        if eng == "A" and a_nm != b_nm:
                            eng = "D"
                    if s == "t4" and last and cfg.get("last_t4_A"):
                        eng = "A"
                    mcol = m_t[:, nfixed + pidx:nfixed + 1 + pidx]
                    if s in merged and not STREAMS[s][2]:
                        # all pieces of this split junk stream write j_gps;
                        # one full-width ts accumulates into the first col
                        # (after the last piece); other cols are zeroed
                        JUNK_s = JUNK
                        JUNK = {k: j_gps for k in JUNK}
                        emit_piece(s, eng, lo, hi, None)
                        JUNK = JUNK_s
                        if s not in first_col:
                            first_col[s] = mcol
                            nc.gpsimd.memset(mcol, 0.0)  # placeholder order
                        else:
                            nc.gpsimd.memset(mcol, 0.0)
                        if hi == IN_F:
                            r = nc.vector.tensor_scalar(
                                j_ts[:, :], j_gps[:, :], 1.0, 0.0, MULT, ADD,
                                accum_out=first_col[s])
                            if cfg.get("gts_prio", 0):
                                r.ins.bass_priority += cfg["gts_prio"]
                        continue
                    if chunked and pidx in wcol_of:
                        wc = wcol_of[pidx]
                        emit_piece(s, eng, 0, H, mcol)
                        emit_piece(s, eng, H, IN_F, m_t[:, wc:wc + 1])
                    else:
                        emit_piece(s, eng, lo, hi, mcol)

                # Transpose moments: [128, K] -> [K, 128] PSUM, copy to SBUF
                mt_ps = pt.tile([mcols, P], F32)
                nc.tensor.transpose(mt_ps[:, :], m_t[:, :], ident[:, :])
                mt_sb = mtsb.tile([mcols, P], MMDT)
                if cfg["mt"] == "D":
                    nc.vector.tensor_copy(mt_sb[:, :], mt_ps[:, :])
                else:
                    nc.scalar.copy(mt_sb[:, :], mt_ps[:, :])

                # out[128, 1024] = MT.T @ GT  (contraction K), f32r
                o_ps = pout.tile([P, OUT_F], F32)
                for h in range(2):
                    nc.tensor.matmul(
                        o_ps[:, h * 512:(h + 1) * 512],
                        lhsT=mt_sb[:, :],
                        rhs=gt_sb[:, h * 512:(h + 1) * 512],
                        start=True, stop=True,
                    )
                o_sb = ostage.tile([P, OUT_F], F32)
                if cfg.get("oc_pipe") and it != NTILES - 1:
                    # pipeline each tile's output: copy half h right after
                    # matmul half h, each half with its own DMA
                    for h in range(2):
                        sl = slice(h * 512, (h + 1) * 512)
                        if (h + it) % 2 == 0:
                            nc.scalar.copy(o_sb[:, sl], o_ps[:, sl])
                        else:
                            nc.vector.tensor_copy(o_sb[:, sl], o_ps[:, sl])
                        nc.sync.dma_start(out=out[it * P:(it + 1) * P, sl],
                                          in_=o_sb[:, sl])
                    continue
                if cfg.get("tail_fast") and it == NTILES - 1:
                    # chunk the last tile's output: copy (alternating
                    # ACT/DVE) + DMA per chunk, pipelined
                    NTC = cfg.get("tail_chunks", 2)
                    QO = OUT_F // NTC
                    for qo in range(NTC):
                        sl = slice(qo * QO, (qo + 1) * QO)
                        if qo % 2 == 0:
                            nc.scalar.copy(o_sb[:, sl], o_ps[:, sl])
                        else:
                            nc.vector.tensor_copy(o_sb[:, sl], o_ps[:, sl])
                        nc.sync.dma_start(out=out[it * P:(it + 1) * P, sl],
                                          in_=o_sb[:, sl])
                    continue
                ca = int(round(OUT_F * cfg["oc_act"] / 128.0)) * 128
                ca = max(0, min(OUT_F, ca))
                ocb = cfg.get("oc_prio", 0)
                if ca > 0:
                    r = nc.scalar.copy(o_sb[:, 0:ca], o_ps[:, 0:ca])
                    if ocb:
                        r.ins.bass_priority += ocb
                if ca < OUT_F:
                    r = nc.vector.tensor_copy(o_sb[:, ca:OUT_F], o_ps[:, ca:OUT_F])
                    if ocb:
                        r.ins.bass_priority += ocb
                nc.sync.dma_start(out=out[it * P:(it + 1) * P, :], in_=o_sb[:, :])

    nc.finalize()
    return nc


_NC_CACHE: dict[tuple, bass.Bass] = {}


def _host_gt(coeffs, cfg=CFG):
    C = _cheb_monomial_matrix()
    G = (coeffs.astype(np.float64) @ C).astype(np.float32)  # [OUT_F, W]
    rows = _moment_rows(cfg)
    GT = np.ascontiguousarray(G.T[rows, :])  # [K, OUT_F]
    return GT


def _run(x, coeffs, input_scale, cfg=CFG, **spmd_kwargs):
    x = np.ascontiguousarray(np.asarray(x, dtype=np.float32))
    coeffs = np.asarray(coeffs, dtype=np.float32)
    scale = float(np.clip(np.asarray(input_scale, dtype=np.float32), 0.1, 2.0).reshape(-1)[0])

    GT = _host_gt(coeffs, cfg)

    key = (scale, str(cfg))
    nc = _NC_CACHE.get(key)
    if nc is None:
        nc = _build_nc(scale, cfg)
        _NC_CACHE[key] = nc

    xs = x.astype(np.float16) if cfg.get("x_f16") else x
    in_maps = [
        {"x": np.ascontiguousarray(xs[c * ROWS_PER_CORE:(c + 1) * ROWS_PER_CORE]),
         "gt": GT}
        for c in range(N_CORES)
    ]
    res = run_bass_kernel_spmd(nc, in_maps, core_ids=list(range(N_CORES)), **spmd_kwargs)
    out = np.concatenate([res.results[c]["out"] for c in range(N_CORES)], axis=0)
    out = out.astype(np.float32)
    if cfg.get("m0_host"):
        C = _cheb_monomial_matrix()
        G = (coeffs.astype(np.float64) @ C)  # [OUT_F, W]
        out = out + (float(IN_F) * G[:, 0]).astype(np.float32)[None, :]
    return out, res


def kernel(x, coeffs, input_scale):
    out, _ = _run(x, coeffs, input_scale)
    return out


if __name__ == "__main__":
    rng = np.random.default_rng(0)
    x = rng.standard_normal((BATCH, IN_F), dtype=np.float32)
    coeffs = (rng.standard_normal((OUT_F, W)) * 0.1).astype(np.float32)
    s = np.ones((1,), np.float32)
    out = kernel(x=x, coeffs=coeffs, input_scale=s)
    print(out.shape, out.dtype)
